# revision 19
# baseline (speedup 1.0000x reference)
"""Trainium2 Bass kernel: single-step BNTM RNN (nn_BNTM_Softmax_7421703487926).

Strategy (8 NeuronCores, SPMD, one NEFF):
  phase A  h = tanh(x @ W_ih.T + b + h_bar @ W_hh.T): column-sharded over HID
           (256 h units per core, interleaved so the post-allgather layout is
           partition-major), h_bar replicated (Wm is tiny).
  comm     AllGather of the 256-float h shards (1 KB/rank).
  phase B  y = sigmoid(Wy h), a = softmax(Wa h), new_elt = sigmoid(Wn h):
           Wy row-sharded (512 outputs/core); Wa/Wn replicated.
  update   memory shift-mix row-sharded (512 rows/core) with halo rows and
           boundary fixups driven by masked per-core constants.

All matvecs run on TensorE with the *vector* as the stationary operand and the
(host-pre-transposed) weight panel as the moving operand, so weights stream
through the PE at one row per cycle and the kernel stays HBM-bound.
"""

import os

import numpy as np

import concourse.bacc as bacc
import concourse.mybir as mybir
import concourse.tile as tile
from concourse.bass_utils import run_bass_kernel_spmd

HID, OUT, VOC, MEM, MDIM, OPN = 2048, 4096, 4096, 4096, 64, 5
NCORES = 8
SH = HID // NCORES          # 256 hidden cols per core
SY = OUT // NCORES          # 512 output rows per core
SM = MEM // NCORES          # 512 memory rows per core
HT = HID // 128             # 16 contraction tiles over HID
XT = VOC // 128             # 32 contraction tiles over VOC
BW = SY + OPN + MDIM        # 581: phase-B fused rhs width
QM = SM // 128              # 4 memory-row chunks per partition

F32 = mybir.dt.float32
MM_DT = mybir.dt.float32    # TensorE compute dtype

AF = mybir.ActivationFunctionType
ALU = mybir.AluOpType


def _c(ap):
    """Bitcast an SBUF operand for TensorE."""
    return ap.bitcast(MM_DT) if MM_DT is not F32 else ap


def _shard_index(r):
    """Global h indices owned by core r, in its local output order n=p'*16+t."""
    p = np.repeat(np.arange(16), 16)
    t = np.tile(np.arange(16), 16)
    return t * 128 + r * 16 + p


def build_nc():
    nc = bacc.Bacc("TRN2", target_bir_lowering=False, debug=False,
                   num_devices=NCORES)

    def inp(name, shape):
        return nc.dram_tensor(name, list(shape), F32, kind="ExternalInput")

    def outp(name, shape):
        return nc.dram_tensor(name, list(shape), F32, kind="ExternalOutput")

    d_xp = inp("xp", (128, XT))
    d_wih = inp("wih", (128, XT * SH))
    d_whh = inp("whh", (128, HT * SH))
    d_bh = inp("bh", (1, SH))
    d_wmt = inp("wmt", (128, HT * MDIM))
    d_mem0b = inp("mem0b", (128, MDIM))
    d_bhb = inp("bhb", (128, HT))
    d_wyt = inp("wyt", (128, HT * BW))
    d_by = inp("by", (1, BW))
    d_mmid = inp("mmid", (128, QM * MDIM))
    d_mnext = inp("mnext", (128, QM * MDIM))
    d_mprev = inp("mprev", (128, QM * MDIM))
    d_fixp = inp("fixp", (1, MDIM))
    d_fixn = inp("fixn", (32, MDIM))
    d_nemask = inp("nemask", (1, MDIM))

    d_y = outp("y", (1, SY))
    d_hc = outp("hc", (1, SH))
    d_mout = outp("mout", (128, QM * MDIM))

    with tile.TileContext(nc) as tc:
        with (
            tc.tile_pool(name="sb", bufs=1) as sb,
            tc.tile_pool(name="pp", bufs=1, space="PSUM") as pp,
            tc.tile_pool(name="dp", bufs=1, space="DRAM") as dp,
        ):
            dma = nc.sync.dma_start
            # second HWDGE issue queue (ACT engine), toggleable for debug
            dma2 = (nc.sync.dma_start if os.environ.get("BNTM_NO_DMA2") == "1"
                    else nc.scalar.dma_start)

            # ---- front-loaded small DMAs (phase-A critical path)
            t_xp = sb.tile([128, XT], F32)
            dma(t_xp[:], d_xp[:])
            t_wmt = sb.tile([128, HT * MDIM], F32)
            dma2(t_wmt[:], d_wmt[:])
            t_m0b = sb.tile([128, MDIM], F32)
            dma2(t_m0b[:], d_mem0b[:])
            t_bhb = sb.tile([128, HT], F32)
            dma2(t_bhb[:], d_bhb[:])
            t_bh = sb.tile([1, SH], F32)
            dma(t_bh[:], d_bh[:])

            # ---- streamed weight panels (host-pretiled: [128, tiles*cols],
            #      fully contiguous per partition; chunked for overlap)
            NCH_IH, CH_IH = 4, XT // 4
            t_wih = []
            for i in range(NCH_IH):
                t = sb.tile([128, CH_IH * SH], F32, name=f"wih{i}")
                (dma if i % 2 == 0 else dma2)(
                    t[:], d_wih[:, i * CH_IH * SH:(i + 1) * CH_IH * SH])
                t_wih.append(t)

            NCH_HH, CH_HH = 2, HT // 2
            t_whh = []
            for i in range(NCH_HH):
                t = sb.tile([128, CH_HH * SH], F32, name=f"whh{i}")
                (dma if i % 2 == 0 else dma2)(
                    t[:], d_whh[:, i * CH_HH * SH:(i + 1) * CH_HH * SH])
                t_whh.append(t)

            NCH_Y, CH_Y = 4, HT // 4
            t_wyt = []
            for i in range(NCH_Y):
                t = sb.tile([128, CH_Y * BW], F32, name=f"wyt{i}")
                (dma if i % 2 == 0 else dma2)(
                    t[:], d_wyt[:, i * CH_Y * BW:(i + 1) * CH_Y * BW])
                t_wyt.append(t)

            t_by = sb.tile([1, BW], F32)
            dma(t_by[:], d_by[:])
            t_mmid = sb.tile([128, QM * MDIM], F32)
            dma2(t_mmid[:], d_mmid[:])
            t_mnext = sb.tile([128, QM * MDIM], F32)
            dma2(t_mnext[:], d_mnext[:])
            t_mprev = sb.tile([128, QM * MDIM], F32)
            dma2(t_mprev[:], d_mprev[:])
            t_fixp = sb.tile([1, MDIM], F32)
            dma(t_fixp[:], d_fixp[:])
            t_fixn = sb.tile([128, MDIM], F32)
            dma(t_fixn[96:128, :], d_fixn[:])
            t_nemask = sb.tile([1, MDIM], F32)
            dma(t_nemask[:], d_nemask[:])

            t_ones = sb.tile([1, 128], F32)
            nc.gpsimd.memset(t_ones[:], 1.0)

            # ---- h_bar = Wm @ mem0 + (Wm_b + hidden0), partition-major.
            # Fused multiply+reduce on VectorE (PE stays free for the x matmuls);
            # the reduce's initial value carries the bias.
            t_hbar = sb.tile([128, HT], F32)
            t_scr = sb.tile([128, MDIM], F32)
            if os.environ.get("BNTM_TTR") == "1":
                # experimental: fused op crashes the device on this runtime
                for t in range(HT):
                    nc.vector.tensor_tensor_reduce(
                        t_scr[:],
                        t_wmt[:, t * MDIM:(t + 1) * MDIM],
                        t_m0b[:],
                        1.0,
                        t_bhb[:, t:t + 1],
                        op0=ALU.mult,
                        op1=ALU.add,
                        accum_out=t_hbar[:, t:t + 1],
                    )
            else:
                for t in range(HT):
                    nc.vector.tensor_mul(t_scr[:],
                                         t_wmt[:, t * MDIM:(t + 1) * MDIM],
                                         t_m0b[:])
                    nc.vector.reduce_sum(t_hbar[:, t:t + 1], t_scr[:],
                                         axis=mybir.AxisListType.X)
                nc.vector.tensor_add(t_hbar[:], t_hbar[:], t_bhb[:])

            # ---- h logits for this core's 256 columns
            ps_h = pp.tile([1, SH], F32)
            nc.tensor.matmul(ps_h[:1, :], _c(t_ones[:1, 0:1]), _c(t_bh[:1, :]),
                             start=True, stop=False)
            for t in range(XT):
                i, j = divmod(t, CH_IH)
                nc.tensor.matmul(ps_h[:1, :], _c(t_xp[:, t:t + 1]),
                                 _c(t_wih[i][:, j * SH:(j + 1) * SH]),
                                 start=False, stop=False)
            for t in range(HT):
                i, j = divmod(t, CH_HH)
                nc.tensor.matmul(ps_h[:1, :], _c(t_hbar[:, t:t + 1]),
                                 _c(t_whh[i][:, j * SH:(j + 1) * SH]),
                                 start=False, stop=(t == HT - 1))

            t_hc = sb.tile([1, SH], F32)
            nc.scalar.activation(t_hc[:1, :], ps_h[:1, :], AF.Tanh)
            dma(d_hc[:], t_hc[:1, :])

            # ---- AllGather the h shards (1 KB per rank)
            cc_in = dp.tile([1, SH], F32)
            cc_out = dp.tile([NCORES, SH], F32, addr_space="Shared")
            dma(cc_in[:], t_hc[:1, :])
            nc.gpsimd.collective_compute(
                "AllGather",
                ALU.bypass,
                replica_groups=[list(range(NCORES))],
                ins=[cc_in[:].opt()],
                outs=[cc_out[:].opt()],
            )
            t_h = sb.tile([128, HT], F32)
            dma(t_h[:], cc_out.rearrange("r (p t) -> (r p) t", p=16))

            # ---- phase B: fused [Wy_shard | Wa | Wn] matvec.  The small
            # (a, new_elt) chain runs first so the softmax + memory update
            # overlap the big Wy matmuls.
            ps_a = pp.tile([1, SY], F32)
            ps_b = pp.tile([1, OPN + MDIM], F32)
            nc.tensor.matmul(ps_b[:1, :], _c(t_ones[:1, 0:1]),
                             _c(t_by[:1, SY:BW]), start=True, stop=False)
            for t in range(HT):
                i, j = divmod(t, CH_Y)
                nc.tensor.matmul(ps_b[:1, :], _c(t_h[:, t:t + 1]),
                                 _c(t_wyt[i][:, j * BW + SY:(j + 1) * BW]),
                                 start=False, stop=(t == HT - 1))
            nc.tensor.matmul(ps_a[:1, :], _c(t_ones[:1, 0:1]),
                             _c(t_by[:1, 0:SY]), start=True, stop=False)
            for t in range(HT):
                i, j = divmod(t, CH_Y)
                nc.tensor.matmul(ps_a[:1, :], _c(t_h[:, t:t + 1]),
                                 _c(t_wyt[i][:, j * BW:j * BW + SY]),
                                 start=False, stop=(t == HT - 1))

            t_y = sb.tile([1, SY], F32)
            nc.scalar.activation(t_y[:1, :], ps_a[:1, :], AF.Sigmoid)
            dma(d_y[:], t_y[:1, :])

            # ---- softmax over the 5 action logits
            t_mx = sb.tile([1, 1], F32)
            nc.vector.reduce_max(t_mx[:1, :], ps_b[:1, 0:OPN],
                                 axis=mybir.AxisListType.X)
            t_nmx = sb.tile([1, 1], F32)
            nc.vector.tensor_scalar_mul(t_nmx[:1, :], t_mx[:1, :], -1.0)
            t_e = sb.tile([1, OPN], F32)
            nc.scalar.activation(t_e[:1, :], ps_b[:1, 0:OPN], AF.Exp,
                                 bias=t_nmx[:1, 0:1])
            t_se = sb.tile([1, 1], F32)
            nc.vector.reduce_sum(t_se[:1, :], t_e[:1, :],
                                 axis=mybir.AxisListType.X)
            t_rs = sb.tile([1, 1], F32)
            nc.vector.reciprocal(t_rs[:1, :], t_se[:1, :])
            t_a = sb.tile([1, OPN], F32)
            nc.vector.tensor_scalar_mul(t_a[:1, :], t_e[:1, :], t_rs[:1, 0:1])

            t_ne = sb.tile([1, MDIM], F32)
            nc.scalar.activation(t_ne[:1, :], ps_b[:1, OPN:OPN + MDIM],
                                 AF.Sigmoid)

            # ---- coefficient vector [a0+a4, a1+a3, a2, -a3, -a4]
            t_cv = sb.tile([1, 5], F32)
            nc.vector.tensor_add(t_cv[:1, 0:1], t_a[:1, 0:1], t_a[:1, 4:5])
            nc.vector.tensor_add(t_cv[:1, 1:2], t_a[:1, 1:2], t_a[:1, 3:4])
            nc.vector.tensor_copy(t_cv[:1, 2:3], t_a[:1, 2:3])
            nc.vector.tensor_scalar_mul(t_cv[:1, 3:4], t_a[:1, 3:4], -1.0)
            nc.vector.tensor_scalar_mul(t_cv[:1, 4:5], t_a[:1, 4:5], -1.0)

            # broadcast coefficients to all 128 partitions via PE rank-1 trick
            ps_bc = pp.tile([128, 5], F32)
            nc.tensor.matmul(ps_bc[:, :], _c(t_ones[:1, :]), _c(t_cv[:1, :]),
                             start=True, stop=True)
            t_cf = sb.tile([128, 5], F32)
            nc.vector.tensor_copy(t_cf[:], ps_bc[:])

            # ---- memory shift-mix: (a0+a4)*next + (a1+a3)*prev + a2*mid
            t_t1 = sb.tile([128, QM * MDIM], F32)
            nc.vector.tensor_scalar_mul(t_t1[:], t_mmid[:], t_cf[:, 2:3])
            t_t2 = sb.tile([128, QM * MDIM], F32)
            nc.vector.scalar_tensor_tensor(t_t2[:], t_mnext[:], t_cf[:, 0:1],
                                           t_t1[:], op0=ALU.mult, op1=ALU.add)
            t_mo = sb.tile([128, QM * MDIM], F32)
            nc.vector.scalar_tensor_tensor(t_mo[:], t_mprev[:], t_cf[:, 1:2],
                                           t_t2[:], op0=ALU.mult, op1=ALU.add)

            # ---- boundary fixups (data-masked; no-ops on non-edge cores)
            # global row 0 (core 0, partition 0, chunk 0):
            #   += new_elt  - a3 * memory[MEM-1]
            nc.vector.scalar_tensor_tensor(
                t_mo[0:1, 0:MDIM], t_fixp[:1, :], t_cf[0:1, 3:4],
                t_mo[0:1, 0:MDIM], op0=ALU.mult, op1=ALU.add)
            t_nem = sb.tile([1, MDIM], F32)
            nc.vector.tensor_mul(t_nem[:1, :], t_ne[:1, :], t_nemask[:1, :])
            nc.vector.tensor_add(t_mo[0:1, 0:MDIM], t_mo[0:1, 0:MDIM],
                                 t_nem[:1, :])
            # global row MEM-1 (core 7, partition 127, last chunk):
            #   -= a4 * memory[0].  fixn is zero except its last row, so the
            #   op runs on the whole [96:128) partition block (DVE ops cannot
            #   start at partition 127) and is a no-op elsewhere.
            nc.vector.scalar_tensor_tensor(
                t_mo[96:128, (QM - 1) * MDIM:QM * MDIM],
                t_fixn[96:128, :], t_cf[96:128, 4:5],
                t_mo[96:128, (QM - 1) * MDIM:QM * MDIM],
                op0=ALU.mult, op1=ALU.add)

            dma(d_mout[:], t_mo[:])

    nc.finalize()
    return nc


_NC_CACHE = None


def _get_nc():
    global _NC_CACHE
    if _NC_CACHE is None:
        _NC_CACHE = build_nc()
    return _NC_CACHE


def make_in_maps(inputs):
    f = lambda a: np.ascontiguousarray(np.asarray(a), dtype=np.float32)
    x = f(inputs["input"]).reshape(VOC)
    h0 = f(inputs["hidden0"]).reshape(HID)
    memory = f(inputs["memory"])
    W_ih, b_ih = f(inputs["W_ih"]), f(inputs["b_ih"])
    W_hh, b_hh = f(inputs["W_hh"]), f(inputs["b_hh"])
    Wm_w, Wm_b = f(inputs["Wm_w"]), f(inputs["Wm_b"])
    Wy_w, Wy_b = f(inputs["Wy_w"]), f(inputs["Wy_b"])
    Wn_w, Wn_b = f(inputs["Wn_w"]), f(inputs["Wn_b"])
    Wa_w, Wa_b = f(inputs["Wa_w"]), f(inputs["Wa_b"])

    def ptile(a, cols):
        """(tiles*128, cols) -> [128, tiles*cols] partition-tiled layout."""
        n = a.shape[0] // 128
        return np.ascontiguousarray(
            a.reshape(n, 128, cols).transpose(1, 0, 2).reshape(128, n * cols))

    xp = np.ascontiguousarray(x.reshape(XT, 128).T)
    wihT = np.ascontiguousarray(W_ih.T)
    whhT = np.ascontiguousarray(W_hh.T)
    wyT = np.ascontiguousarray(Wy_w.T)
    waT = np.ascontiguousarray(Wa_w.T)
    wnT = np.ascontiguousarray(Wn_w.T)
    wm_tiled = ptile(Wm_w, MDIM)
    mem0b = np.ascontiguousarray(np.tile(memory[0], (128, 1)))
    bh_full = b_ih + b_hh
    bhb = np.ascontiguousarray((Wm_b + h0).reshape(HT, 128).T)
    zeros64 = np.zeros(MDIM, np.float32)
    ones64 = np.ones(MDIM, np.float32)
    fixn_r7 = np.zeros((32, MDIM), np.float32)
    fixn_r7[31] = memory[0]

    def rearr(a):
        return np.ascontiguousarray(
            a.reshape(QM, 128, MDIM).transpose(1, 0, 2).reshape(128, QM * MDIM))

    in_maps = []
    for r in range(NCORES):
        idx = _shard_index(r)
        rows = np.arange(r * SM, (r + 1) * SM)
        im = {
            "xp": xp,
            "wih": ptile(wihT[:, idx], SH),
            "whh": ptile(whhT[:, idx], SH),
            "bh": bh_full[idx].reshape(1, SH),
            "wmt": wm_tiled,
            "mem0b": mem0b,
            "bhb": bhb,
            "wyt": ptile(np.concatenate(
                [wyT[:, r * SY:(r + 1) * SY], waT, wnT], axis=1), BW),
            "by": np.concatenate(
                [Wy_b[r * SY:(r + 1) * SY], Wa_b, Wn_b]).reshape(1, BW),
            "mmid": rearr(memory[rows]),
            "mnext": rearr(memory[(rows + 1) % MEM]),
            "mprev": rearr(memory[(rows - 1) % MEM]),
            "fixp": (memory[MEM - 1] if r == 0 else zeros64).reshape(1, MDIM),
            "fixn": fixn_r7 if r == NCORES - 1 else np.zeros((32, MDIM),
                                                            np.float32),
            "nemask": (ones64 if r == 0 else zeros64).reshape(1, MDIM),
        }
        in_maps.append({k: np.ascontiguousarray(v, dtype=np.float32)
                        for k, v in im.items()})
    return in_maps


def assemble_outputs(results):
    y = np.concatenate(
        [results[r]["y"].reshape(SY) for r in range(NCORES)]).reshape(1, 1, OUT)
    ht = np.empty(HID, np.float32)
    for r in range(NCORES):
        ht[_shard_index(r)] = results[r]["hc"].reshape(SH)
    ht = ht.reshape(1, 1, HID)
    mem = np.concatenate(
        [results[r]["mout"].reshape(128, QM, MDIM).transpose(1, 0, 2)
         .reshape(SM, MDIM) for r in range(NCORES)], axis=0)
    return (np.ascontiguousarray(y, np.float32),
            np.ascontiguousarray(ht, np.float32),
            np.ascontiguousarray(mem, np.float32))


def kernel(**inputs):
    nc = _get_nc()
    in_maps = make_in_maps(inputs)
    res = run_bass_kernel_spmd(nc, in_maps, list(range(NCORES))).results
    return assemble_outputs(res)


# revision 28
# speedup vs baseline: 1.1409x; 1.1409x over previous
"""Trainium2 Bass kernel: single-step BNTM RNN (nn_BNTM_Softmax_7421703487926).

Strategy (8 NeuronCores, SPMD, one NEFF):
  phase A  h = tanh(x @ W_ih.T + b + h_bar @ W_hh.T): column-sharded over HID
           (256 h units per core, interleaved so the post-allgather layout is
           partition-major), h_bar replicated (Wm is tiny).
  comm     AllGather of the 256-float h shards (1 KB/rank).
  phase B  y = sigmoid(Wy h), a = softmax(Wa h), new_elt = sigmoid(Wn h):
           Wy row-sharded (512 outputs/core); Wa/Wn replicated.
  update   memory shift-mix row-sharded (512 rows/core) with halo rows and
           boundary fixups driven by masked per-core constants.

All matvecs run on TensorE with the *vector* as the stationary operand and the
(host-pre-transposed) weight panel as the moving operand, so weights stream
through the PE at one row per cycle and the kernel stays HBM-bound.
"""

import os

import numpy as np

import concourse.bacc as bacc
import concourse.mybir as mybir
import concourse.tile as tile
from concourse.bass_utils import run_bass_kernel_spmd

HID, OUT, VOC, MEM, MDIM, OPN = 2048, 4096, 4096, 4096, 64, 5
NCORES = 8
SH = HID // NCORES          # 256 hidden cols per core
SY = OUT // NCORES          # 512 output rows per core
SM = MEM // NCORES          # 512 memory rows per core
HT = HID // 128             # 16 contraction tiles over HID
XT = VOC // 128             # 32 contraction tiles over VOC
BW = SY + OPN + MDIM + 1    # 582: phase-B fused rhs width (even-padded for f32r)
QM = SM // 128              # 4 memory-row chunks per partition

F32 = mybir.dt.float32
# TensorE compute dtype: float32r streams 1 row/cycle (vs 4 for float32) at
# ~1e-4 relative error (TF32-class).  BNTM_F32R=0 falls back to exact fp32.
MM_DT = (mybir.dt.float32 if os.environ.get("BNTM_F32R") == "0"
         else mybir.dt.float32r)

AF = mybir.ActivationFunctionType
ALU = mybir.AluOpType


def _shard_index(r):
    """Global h indices owned by core r, in its local output order n=p'*16+t."""
    p = np.repeat(np.arange(16), 16)
    t = np.tile(np.arange(16), 16)
    return t * 128 + r * 16 + p


def build_nc():
    nc = bacc.Bacc("TRN2", target_bir_lowering=False, debug=False,
                   num_devices=NCORES)

    def inp(name, shape, dt=F32):
        return nc.dram_tensor(name, list(shape), dt, kind="ExternalInput")

    def outp(name, shape):
        return nc.dram_tensor(name, list(shape), F32, kind="ExternalOutput")

    d_xp = inp("xp", (128, XT), MM_DT)
    d_wih = inp("wih", (128, XT * SH), MM_DT)
    d_whh = inp("whh", (128, HT * SH), MM_DT)
    d_bh = inp("bh", (1, SH), MM_DT)
    d_wmt = inp("wmt", (128, HT * MDIM))
    d_mem0b = inp("mem0b", (128, MDIM))
    d_bhb = inp("bhb", (128, HT))
    d_wyt = inp("wyt", (128, HT * BW), MM_DT)
    d_by = inp("by", (1, BW), MM_DT)
    d_ones = inp("ones", (1, 128), MM_DT)
    d_mmid = inp("mmid", (128, QM * MDIM))
    d_mnext = inp("mnext", (128, QM * MDIM))
    d_mprev = inp("mprev", (128, QM * MDIM))
    d_fixp = inp("fixp", (1, MDIM))
    d_fixn = inp("fixn", (32, MDIM))
    d_nemask = inp("nemask", (1, MDIM))

    d_y = outp("y", (1, SY))
    d_hc = outp("hc", (1, SH))
    d_mout = outp("mout", (128, QM * MDIM))

    with tile.TileContext(nc) as tc:
        with (
            tc.tile_pool(name="sb", bufs=1) as sb,
            tc.tile_pool(name="pp", bufs=1, space="PSUM") as pp,
            tc.tile_pool(name="dp", bufs=1, space="DRAM") as dp,
        ):
            dma = nc.sync.dma_start
            # second HWDGE issue queue (ACT engine), toggleable for debug
            dma2 = (nc.sync.dma_start if os.environ.get("BNTM_NO_DMA2") == "1"
                    else nc.scalar.dma_start)

            # ---- front-loaded small DMAs (phase-A critical path)
            t_xp = sb.tile([128, XT], MM_DT)
            dma(t_xp[:], d_xp[:])
            t_wmt = sb.tile([128, HT * MDIM], F32)
            dma2(t_wmt[:], d_wmt[:])
            t_m0b = sb.tile([128, MDIM], F32)
            dma2(t_m0b[:], d_mem0b[:])
            t_bhb = sb.tile([128, HT], F32)
            dma2(t_bhb[:], d_bhb[:])
            t_bh = sb.tile([1, SH], MM_DT)
            dma(t_bh[:], d_bh[:])

            # ---- streamed weight panels (host-pretiled: [128, tiles*cols],
            #      fully contiguous per partition; chunked for overlap)
            NCH_IH, CH_IH = 4, XT // 4
            t_wih = []
            for i in range(NCH_IH):
                t = sb.tile([128, CH_IH * SH], MM_DT, name=f"wih{i}")
                (dma if i % 2 == 0 else dma2)(
                    t[:], d_wih[:, i * CH_IH * SH:(i + 1) * CH_IH * SH])
                t_wih.append(t)

            NCH_HH, CH_HH = 2, HT // 2
            t_whh = []
            for i in range(NCH_HH):
                t = sb.tile([128, CH_HH * SH], MM_DT, name=f"whh{i}")
                (dma if i % 2 == 0 else dma2)(
                    t[:], d_whh[:, i * CH_HH * SH:(i + 1) * CH_HH * SH])
                t_whh.append(t)

            NCH_Y, CH_Y = 4, HT // 4
            t_wyt = []
            for i in range(NCH_Y):
                t = sb.tile([128, CH_Y * BW], MM_DT, name=f"wyt{i}")
                (dma if i % 2 == 0 else dma2)(
                    t[:], d_wyt[:, i * CH_Y * BW:(i + 1) * CH_Y * BW])
                t_wyt.append(t)

            t_by = sb.tile([1, BW], MM_DT)
            dma(t_by[:], d_by[:])
            t_mmid = sb.tile([128, QM * MDIM], F32)
            dma2(t_mmid[:], d_mmid[:])
            t_mnext = sb.tile([128, QM * MDIM], F32)
            dma2(t_mnext[:], d_mnext[:])
            t_mprev = sb.tile([128, QM * MDIM], F32)
            dma2(t_mprev[:], d_mprev[:])
            t_fixp = sb.tile([1, MDIM], F32)
            dma(t_fixp[:], d_fixp[:])
            t_fixn = sb.tile([128, MDIM], F32)
            dma(t_fixn[96:128, :], d_fixn[:])
            t_nemask = sb.tile([1, MDIM], F32)
            dma(t_nemask[:], d_nemask[:])

            t_ones = sb.tile([1, 128], MM_DT)
            dma(t_ones[:], d_ones[:])
            # prefetch the Tanh LUT while ScalarE is idle (real tanh sits on
            # the phase-A critical path)
            t_dummy = sb.tile([1, 1], F32)
            nc.scalar.activation(t_dummy[:1, :1], t_bhb[0:1, 0:1], AF.Tanh)

            # ---- h_bar = Wm @ mem0 + (Wm_b + hidden0), partition-major.
            # Fused multiply+reduce on VectorE (PE stays free for the x matmuls);
            # the reduce's initial value carries the bias.
            t_hbar = sb.tile([128, HT], F32)
            t_scr = sb.tile([128, MDIM], F32)
            if os.environ.get("BNTM_TTR") == "1":
                # experimental: fused op crashes the device on this runtime
                for t in range(HT):
                    nc.vector.tensor_tensor_reduce(
                        t_scr[:],
                        t_wmt[:, t * MDIM:(t + 1) * MDIM],
                        t_m0b[:],
                        1.0,
                        t_bhb[:, t:t + 1],
                        op0=ALU.mult,
                        op1=ALU.add,
                        accum_out=t_hbar[:, t:t + 1],
                    )
            else:
                for t in range(HT):
                    nc.vector.tensor_mul(t_scr[:],
                                         t_wmt[:, t * MDIM:(t + 1) * MDIM],
                                         t_m0b[:])
                    nc.vector.reduce_sum(t_hbar[:, t:t + 1], t_scr[:],
                                         axis=mybir.AxisListType.X)
                nc.vector.tensor_add(t_hbar[:], t_hbar[:], t_bhb[:])
            t_hbr = sb.tile([128, HT], MM_DT)
            nc.vector.tensor_copy(t_hbr[:], t_hbar[:])

            # ---- h logits for this core's 256 columns
            ps_h = pp.tile([1, SH], F32)
            nc.tensor.matmul(ps_h[:1, :], t_ones[:1, 0:1], t_bh[:1, :],
                             start=True, stop=False)
            for t in range(XT):
                i, j = divmod(t, CH_IH)
                nc.tensor.matmul(ps_h[:1, :], t_xp[:, t:t + 1],
                                 t_wih[i][:, j * SH:(j + 1) * SH],
                                 start=False, stop=False)
            for t in range(HT):
                i, j = divmod(t, CH_HH)
                nc.tensor.matmul(ps_h[:1, :], t_hbr[:, t:t + 1],
                                 t_whh[i][:, j * SH:(j + 1) * SH],
                                 start=False, stop=(t == HT - 1))

            t_hc = sb.tile([1, SH], F32)
            nc.scalar.activation(t_hc[:1, :], ps_h[:1, :], AF.Tanh)

            # ---- AllGather the h shards (1 KB per rank)
            cc_in = dp.tile([1, SH], MM_DT)
            cc_out = dp.tile([NCORES, SH], MM_DT, addr_space="Shared")
            dma(cc_in[:], t_hc[:1, :].bitcast(MM_DT))
            dma(d_hc[:], t_hc[:1, :])
            nc.gpsimd.collective_compute(
                "AllGather",
                ALU.bypass,
                replica_groups=[list(range(NCORES))],
                ins=[cc_in[:].opt()],
                outs=[cc_out[:].opt()],
            )
            t_h = sb.tile([128, HT], MM_DT)
            dma(t_h[:], cc_out.rearrange("r (p t) -> (r p) t", p=16))

            # ---- phase B: fused [Wy_shard | Wa | Wn] matvec.  The small
            # (a, new_elt) chain runs first so the softmax + memory update
            # overlap the big Wy matmuls.
            ps_a = pp.tile([1, SY], F32)
            ps_b = pp.tile([1, OPN + MDIM + 1], F32)
            nc.tensor.matmul(ps_b[:1, :], t_ones[:1, 0:1],
                             t_by[:1, SY:BW], start=True, stop=False)
            nc.tensor.matmul(ps_a[:1, :], t_ones[:1, 0:1],
                             t_by[:1, 0:SY], start=True, stop=False)
            for t in range(HT):
                i, j = divmod(t, CH_Y)
                nc.tensor.matmul(ps_b[:1, :], t_h[:, t:t + 1],
                                 t_wyt[i][:, j * BW + SY:(j + 1) * BW],
                                 start=False, stop=(t == HT - 1))

            # ---- softmax over the 5 action logits
            t_mx = sb.tile([1, 1], F32)
            nc.vector.reduce_max(t_mx[:1, :], ps_b[:1, 0:OPN],
                                 axis=mybir.AxisListType.X)
            t_nmx = sb.tile([1, 1], F32)
            nc.vector.tensor_scalar_mul(t_nmx[:1, :], t_mx[:1, :], -1.0)
            t_e = sb.tile([1, OPN], F32)
            nc.scalar.activation(t_e[:1, :], ps_b[:1, 0:OPN], AF.Exp,
                                 bias=t_nmx[:1, 0:1])
            t_se = sb.tile([1, 1], F32)
            nc.vector.reduce_sum(t_se[:1, :], t_e[:1, :],
                                 axis=mybir.AxisListType.X)
            t_rs = sb.tile([1, 1], F32)
            nc.vector.reciprocal(t_rs[:1, :], t_se[:1, :])
            t_a = sb.tile([1, OPN], F32)
            nc.vector.tensor_scalar_mul(t_a[:1, :], t_e[:1, :], t_rs[:1, 0:1])

            t_ne = sb.tile([1, MDIM], F32)
            nc.scalar.activation(t_ne[:1, :], ps_b[:1, OPN:OPN + MDIM],
                                 AF.Sigmoid)

            # ---- coefficient vector [a0+a4, a1+a3, a2, -a3, -a4]
            t_cv = sb.tile([1, 5], F32)
            nc.vector.tensor_add(t_cv[:1, 0:1], t_a[:1, 0:1], t_a[:1, 4:5])
            nc.vector.tensor_add(t_cv[:1, 1:2], t_a[:1, 1:2], t_a[:1, 3:4])
            nc.vector.tensor_copy(t_cv[:1, 2:3], t_a[:1, 2:3])
            nc.vector.tensor_scalar_mul(t_cv[:1, 3:4], t_a[:1, 3:4], -1.0)
            nc.vector.tensor_scalar_mul(t_cv[:1, 4:5], t_a[:1, 4:5], -1.0)

            # broadcast coefficients to all 128 partitions via PE rank-1 trick
            ps_bc = pp.tile([128, 5], F32)
            nc.tensor.matmul(ps_bc[:, :], t_ones[:1, :].bitcast(F32),
                             t_cv[:1, :], start=True, stop=True)
            t_cf = sb.tile([128, 5], F32)
            nc.vector.tensor_copy(t_cf[:], ps_bc[:])

            # ---- memory shift-mix: (a0+a4)*next + (a1+a3)*prev + a2*mid
            t_t1 = sb.tile([128, QM * MDIM], F32)
            nc.vector.tensor_scalar_mul(t_t1[:], t_mmid[:], t_cf[:, 2:3])
            t_t2 = sb.tile([128, QM * MDIM], F32)
            nc.vector.scalar_tensor_tensor(t_t2[:], t_mnext[:], t_cf[:, 0:1],
                                           t_t1[:], op0=ALU.mult, op1=ALU.add)
            t_mo = sb.tile([128, QM * MDIM], F32)
            nc.vector.scalar_tensor_tensor(t_mo[:], t_mprev[:], t_cf[:, 1:2],
                                           t_t2[:], op0=ALU.mult, op1=ALU.add)

            # ---- boundary fixups (data-masked; no-ops on non-edge cores)
            # global row 0 (core 0, partition 0, chunk 0):
            #   += new_elt  - a3 * memory[MEM-1]
            nc.vector.scalar_tensor_tensor(
                t_mo[0:1, 0:MDIM], t_fixp[:1, :], t_cf[0:1, 3:4],
                t_mo[0:1, 0:MDIM], op0=ALU.mult, op1=ALU.add)
            t_nem = sb.tile([1, MDIM], F32)
            nc.vector.tensor_mul(t_nem[:1, :], t_ne[:1, :], t_nemask[:1, :])
            nc.vector.tensor_add(t_mo[0:1, 0:MDIM], t_mo[0:1, 0:MDIM],
                                 t_nem[:1, :])
            # global row MEM-1 (core 7, partition 127, last chunk):
            #   -= a4 * memory[0].  fixn is zero except its last row, so the
            #   op runs on the whole [96:128) partition block (DVE ops cannot
            #   start at partition 127) and is a no-op elsewhere.
            nc.vector.scalar_tensor_tensor(
                t_mo[96:128, (QM - 1) * MDIM:QM * MDIM],
                t_fixn[96:128, :], t_cf[96:128, 4:5],
                t_mo[96:128, (QM - 1) * MDIM:QM * MDIM],
                op0=ALU.mult, op1=ALU.add)

            for t in range(HT):
                i, j = divmod(t, CH_Y)
                nc.tensor.matmul(ps_a[:1, :], t_h[:, t:t + 1],
                                 t_wyt[i][:, j * BW:j * BW + SY],
                                 start=False, stop=(t == HT - 1))
            t_y = sb.tile([1, SY], F32)
            nc.scalar.activation(t_y[:1, :], ps_a[:1, :], AF.Sigmoid)
            dma(d_y[:], t_y[:1, :])

            dma(d_mout[:], t_mo[:])

    nc.finalize()
    return nc


_NC_CACHE = None


def _get_nc():
    global _NC_CACHE
    if _NC_CACHE is None:
        _NC_CACHE = build_nc()
    return _NC_CACHE


def make_in_maps(inputs):
    f = lambda a: np.ascontiguousarray(np.asarray(a), dtype=np.float32)
    x = f(inputs["input"]).reshape(VOC)
    h0 = f(inputs["hidden0"]).reshape(HID)
    memory = f(inputs["memory"])
    W_ih, b_ih = f(inputs["W_ih"]), f(inputs["b_ih"])
    W_hh, b_hh = f(inputs["W_hh"]), f(inputs["b_hh"])
    Wm_w, Wm_b = f(inputs["Wm_w"]), f(inputs["Wm_b"])
    Wy_w, Wy_b = f(inputs["Wy_w"]), f(inputs["Wy_b"])
    Wn_w, Wn_b = f(inputs["Wn_w"]), f(inputs["Wn_b"])
    Wa_w, Wa_b = f(inputs["Wa_w"]), f(inputs["Wa_b"])

    def ptile(a, cols):
        """(tiles*128, cols) -> [128, tiles*cols] partition-tiled layout."""
        n = a.shape[0] // 128
        return np.ascontiguousarray(
            a.reshape(n, 128, cols).transpose(1, 0, 2).reshape(128, n * cols))

    xp = np.ascontiguousarray(x.reshape(XT, 128).T)
    wihT = np.ascontiguousarray(W_ih.T)
    whhT = np.ascontiguousarray(W_hh.T)
    wyT = np.ascontiguousarray(Wy_w.T)
    waT = np.ascontiguousarray(Wa_w.T)
    wnT = np.ascontiguousarray(Wn_w.T)
    wm_tiled = ptile(Wm_w, MDIM)
    mem0b = np.ascontiguousarray(np.tile(memory[0], (128, 1)))
    bh_full = b_ih + b_hh
    bhb = np.ascontiguousarray((Wm_b + h0).reshape(HT, 128).T)
    zeros64 = np.zeros(MDIM, np.float32)
    ones64 = np.ones(MDIM, np.float32)
    fixn_r7 = np.zeros((32, MDIM), np.float32)
    fixn_r7[31] = memory[0]

    def rearr(a):
        return np.ascontiguousarray(
            a.reshape(QM, 128, MDIM).transpose(1, 0, 2).reshape(128, QM * MDIM))

    in_maps = []
    for r in range(NCORES):
        idx = _shard_index(r)
        rows = np.arange(r * SM, (r + 1) * SM)
        im = {
            "ones": np.ones((1, 128), np.float32),
            "xp": xp,
            "wih": ptile(wihT[:, idx], SH),
            "whh": ptile(whhT[:, idx], SH),
            "bh": bh_full[idx].reshape(1, SH),
            "wmt": wm_tiled,
            "mem0b": mem0b,
            "bhb": bhb,
            "wyt": ptile(np.concatenate(
                [wyT[:, r * SY:(r + 1) * SY], waT, wnT,
                 np.zeros((HID, 1), np.float32)], axis=1), BW),
            "by": np.concatenate(
                [Wy_b[r * SY:(r + 1) * SY], Wa_b, Wn_b,
                 np.zeros(1, np.float32)]).reshape(1, BW),
            "mmid": rearr(memory[rows]),
            "mnext": rearr(memory[(rows + 1) % MEM]),
            "mprev": rearr(memory[(rows - 1) % MEM]),
            "fixp": (memory[MEM - 1] if r == 0 else zeros64).reshape(1, MDIM),
            "fixn": fixn_r7 if r == NCORES - 1 else np.zeros((32, MDIM),
                                                            np.float32),
            "nemask": (ones64 if r == 0 else zeros64).reshape(1, MDIM),
        }
        in_maps.append({k: np.ascontiguousarray(v, dtype=np.float32)
                        for k, v in im.items()})
    return in_maps


def assemble_outputs(results):
    y = np.concatenate(
        [results[r]["y"].reshape(SY) for r in range(NCORES)]).reshape(1, 1, OUT)
    ht = np.empty(HID, np.float32)
    for r in range(NCORES):
        ht[_shard_index(r)] = results[r]["hc"].reshape(SH)
    ht = ht.reshape(1, 1, HID)
    mem = np.concatenate(
        [results[r]["mout"].reshape(128, QM, MDIM).transpose(1, 0, 2)
         .reshape(SM, MDIM) for r in range(NCORES)], axis=0)
    return (np.ascontiguousarray(y, np.float32),
            np.ascontiguousarray(ht, np.float32),
            np.ascontiguousarray(mem, np.float32))


def kernel(**inputs):
    nc = _get_nc()
    in_maps = make_in_maps(inputs)
    res = run_bass_kernel_spmd(nc, in_maps, list(range(NCORES))).results
    return assemble_outputs(res)


# revision 29
# speedup vs baseline: 1.1847x; 1.0384x over previous
"""Trainium2 Bass kernel: single-step BNTM RNN (nn_BNTM_Softmax_7421703487926).

Strategy (8 NeuronCores, SPMD, one NEFF):
  phase A  h = tanh(x @ W_ih.T + b + h_bar @ W_hh.T): column-sharded over HID
           (256 h units per core, interleaved so the post-allgather layout is
           partition-major), h_bar replicated (Wm is tiny).
  comm     AllGather of the 256-float h shards (1 KB/rank).
  phase B  y = sigmoid(Wy h), a = softmax(Wa h), new_elt = sigmoid(Wn h):
           Wy row-sharded (512 outputs/core); Wa/Wn replicated.
  update   memory shift-mix row-sharded (512 rows/core) with halo rows and
           boundary fixups driven by masked per-core constants.

All matvecs run on TensorE with the *vector* as the stationary operand and the
(host-pre-transposed) weight panel as the moving operand, so weights stream
through the PE at one row per cycle and the kernel stays HBM-bound.
"""

import os

import numpy as np

import concourse.bacc as bacc
import concourse.mybir as mybir
import concourse.tile as tile
from concourse.bass_utils import run_bass_kernel_spmd

HID, OUT, VOC, MEM, MDIM, OPN = 2048, 4096, 4096, 4096, 64, 5
NCORES = 8
SH = HID // NCORES          # 256 hidden cols per core
SY = OUT // NCORES          # 512 output rows per core
SM = MEM // NCORES          # 512 memory rows per core
HT = HID // 128             # 16 contraction tiles over HID
XT = VOC // 128             # 32 contraction tiles over VOC
BW = SY + OPN + MDIM + 1    # 582: phase-B fused rhs width (even-padded for f32r)
QM = SM // 128              # 4 memory-row chunks per partition

F32 = mybir.dt.float32
# TensorE compute dtype: float32r streams 1 row/cycle (vs 4 for float32) at
# ~1e-4 relative error (TF32-class).  BNTM_F32R=0 falls back to exact fp32.
MM_DT = (mybir.dt.float32 if os.environ.get("BNTM_F32R") == "0"
         else mybir.dt.float32r)

AF = mybir.ActivationFunctionType
ALU = mybir.AluOpType


def _shard_index(r):
    """Global h indices owned by core r, in its local output order n=p'*16+t."""
    p = np.repeat(np.arange(16), 16)
    t = np.tile(np.arange(16), 16)
    return t * 128 + r * 16 + p


def build_nc():
    nc = bacc.Bacc("TRN2", target_bir_lowering=False, debug=False,
                   num_devices=NCORES)

    def inp(name, shape, dt=F32):
        return nc.dram_tensor(name, list(shape), dt, kind="ExternalInput")

    def outp(name, shape):
        return nc.dram_tensor(name, list(shape), F32, kind="ExternalOutput")

    d_xp = inp("xp", (128, XT), MM_DT)
    d_wih = inp("wih", (128, XT * SH), MM_DT)
    d_whh = inp("whh", (128, HT * SH), MM_DT)
    d_bh = inp("bh", (1, SH), MM_DT)
    d_wmt = inp("wmt", (128, HT * MDIM))
    d_mem0b = inp("mem0b", (128, MDIM))
    d_bhb = inp("bhb", (128, HT))
    d_wyt = inp("wyt", (128, HT * BW), MM_DT)
    d_by = inp("by", (1, BW), MM_DT)
    d_ones = inp("ones", (1, 128), MM_DT)
    d_mmid = inp("mmid", (128, QM * MDIM))
    d_mnext = inp("mnext", (128, QM * MDIM))
    d_mprev = inp("mprev", (128, QM * MDIM))
    d_fixp = inp("fixp", (1, MDIM))
    d_fixn = inp("fixn", (32, MDIM))
    d_nemask = inp("nemask", (1, MDIM))

    d_y = outp("y", (1, SY))
    d_hc = outp("hc", (1, SH))
    d_mout = outp("mout", (128, QM * MDIM))

    with tile.TileContext(nc) as tc:
        with (
            tc.tile_pool(name="sb", bufs=1) as sb,
            tc.tile_pool(name="pp", bufs=1, space="PSUM") as pp,
            tc.tile_pool(name="dp", bufs=1, space="DRAM") as dp,
        ):
            dma = nc.sync.dma_start
            # second HWDGE issue queue (ACT engine), toggleable for debug
            dma2 = (nc.sync.dma_start if os.environ.get("BNTM_NO_DMA2") == "1"
                    else nc.scalar.dma_start)

            # ---- front-loaded small DMAs (phase-A critical path)
            t_xp = sb.tile([128, XT], MM_DT)
            dma(t_xp[:], d_xp[:])
            t_wmt = sb.tile([128, HT * MDIM], F32)
            dma2(t_wmt[:], d_wmt[:])
            t_m0b = sb.tile([128, MDIM], F32)
            dma2(t_m0b[:], d_mem0b[:])
            t_bhb = sb.tile([128, HT], F32)
            dma2(t_bhb[:], d_bhb[:])
            t_bh = sb.tile([1, SH], MM_DT)
            dma(t_bh[:], d_bh[:])
            t_ones = sb.tile([1, 128], MM_DT)
            dma(t_ones[:], d_ones[:])
            t_by = sb.tile([1, BW], MM_DT)
            dma(t_by[:], d_by[:])
            t_fixp = sb.tile([1, MDIM], F32)
            dma(t_fixp[:], d_fixp[:])
            t_fixn = sb.tile([128, MDIM], F32)
            dma(t_fixn[96:128, :], d_fixn[:])
            t_nemask = sb.tile([1, MDIM], F32)
            dma(t_nemask[:], d_nemask[:])

            # ---- streamed weight panels (host-pretiled: [128, tiles*cols],
            #      fully contiguous per partition; chunked for overlap)
            NCH_IH, CH_IH = 4, XT // 4
            t_wih = []
            for i in range(NCH_IH):
                t = sb.tile([128, CH_IH * SH], MM_DT, name=f"wih{i}")
                (dma if i % 2 == 0 else dma2)(
                    t[:], d_wih[:, i * CH_IH * SH:(i + 1) * CH_IH * SH])
                t_wih.append(t)

            NCH_HH, CH_HH = 2, HT // 2
            t_whh = []
            for i in range(NCH_HH):
                t = sb.tile([128, CH_HH * SH], MM_DT, name=f"whh{i}")
                (dma if i % 2 == 0 else dma2)(
                    t[:], d_whh[:, i * CH_HH * SH:(i + 1) * CH_HH * SH])
                t_whh.append(t)

            NCH_Y, CH_Y = 4, HT // 4
            t_wyt = []
            for i in range(NCH_Y):
                t = sb.tile([128, CH_Y * BW], MM_DT, name=f"wyt{i}")
                (dma if i % 2 == 0 else dma2)(
                    t[:], d_wyt[:, i * CH_Y * BW:(i + 1) * CH_Y * BW])
                t_wyt.append(t)

            t_mmid = sb.tile([128, QM * MDIM], F32)
            dma2(t_mmid[:], d_mmid[:])
            t_mnext = sb.tile([128, QM * MDIM], F32)
            dma2(t_mnext[:], d_mnext[:])
            t_mprev = sb.tile([128, QM * MDIM], F32)
            dma2(t_mprev[:], d_mprev[:])

            # prefetch the Tanh LUT while ScalarE is idle (real tanh sits on
            # the phase-A critical path)
            t_dummy = sb.tile([1, 1], F32)
            nc.scalar.activation(t_dummy[:1, :1], t_bhb[0:1, 0:1], AF.Tanh)

            # ---- h_bar = Wm @ mem0 + (Wm_b + hidden0), partition-major.
            # Fused multiply+reduce on VectorE (PE stays free for the x matmuls);
            # the reduce's initial value carries the bias.
            t_hbar = sb.tile([128, HT], F32)
            t_scr = sb.tile([128, MDIM], F32)
            if os.environ.get("BNTM_TTR") == "1":
                # experimental: fused op crashes the device on this runtime
                for t in range(HT):
                    nc.vector.tensor_tensor_reduce(
                        t_scr[:],
                        t_wmt[:, t * MDIM:(t + 1) * MDIM],
                        t_m0b[:],
                        1.0,
                        t_bhb[:, t:t + 1],
                        op0=ALU.mult,
                        op1=ALU.add,
                        accum_out=t_hbar[:, t:t + 1],
                    )
            else:
                for t in range(HT):
                    nc.vector.tensor_mul(t_scr[:],
                                         t_wmt[:, t * MDIM:(t + 1) * MDIM],
                                         t_m0b[:])
                    nc.vector.reduce_sum(t_hbar[:, t:t + 1], t_scr[:],
                                         axis=mybir.AxisListType.X)
                nc.vector.tensor_add(t_hbar[:], t_hbar[:], t_bhb[:])
            t_hbr = sb.tile([128, HT], MM_DT)
            nc.vector.tensor_copy(t_hbr[:], t_hbar[:])

            # ---- h logits for this core's 256 columns
            ps_h = pp.tile([1, SH], F32)
            nc.tensor.matmul(ps_h[:1, :], t_ones[:1, 0:1], t_bh[:1, :],
                             start=True, stop=False)
            for t in range(XT):
                i, j = divmod(t, CH_IH)
                nc.tensor.matmul(ps_h[:1, :], t_xp[:, t:t + 1],
                                 t_wih[i][:, j * SH:(j + 1) * SH],
                                 start=False, stop=False)
            for t in range(HT):
                i, j = divmod(t, CH_HH)
                nc.tensor.matmul(ps_h[:1, :], t_hbr[:, t:t + 1],
                                 t_whh[i][:, j * SH:(j + 1) * SH],
                                 start=False, stop=(t == HT - 1))

            t_hc = sb.tile([1, SH], F32)
            nc.scalar.activation(t_hc[:1, :], ps_h[:1, :], AF.Tanh)

            # ---- AllGather the h shards (1 KB per rank)
            cc_in = dp.tile([1, SH], MM_DT)
            cc_out = dp.tile([NCORES, SH], MM_DT, addr_space="Shared")
            dma(cc_in[:], t_hc[:1, :].bitcast(MM_DT))
            dma(d_hc[:], t_hc[:1, :])
            nc.gpsimd.collective_compute(
                "AllGather",
                ALU.bypass,
                replica_groups=[list(range(NCORES))],
                ins=[cc_in[:].opt()],
                outs=[cc_out[:].opt()],
            )
            t_h = sb.tile([128, HT], MM_DT)
            dma(t_h[:], cc_out.rearrange("r (p t) -> (r p) t", p=16))

            # ---- phase B: fused [Wy_shard | Wa | Wn] matvec.  The small
            # (a, new_elt) chain runs first so the softmax + memory update
            # overlap the big Wy matmuls.
            ps_a = pp.tile([1, SY], F32)
            ps_b = pp.tile([1, OPN + MDIM + 1], F32)
            nc.tensor.matmul(ps_b[:1, :], t_ones[:1, 0:1],
                             t_by[:1, SY:BW], start=True, stop=False)
            nc.tensor.matmul(ps_a[:1, :], t_ones[:1, 0:1],
                             t_by[:1, 0:SY], start=True, stop=False)
            for t in range(HT):
                i, j = divmod(t, CH_Y)
                nc.tensor.matmul(ps_b[:1, :], t_h[:, t:t + 1],
                                 t_wyt[i][:, j * BW + SY:(j + 1) * BW],
                                 start=False, stop=(t == HT - 1))

            # ---- softmax over the 5 action logits
            t_mx = sb.tile([1, 1], F32)
            nc.vector.reduce_max(t_mx[:1, :], ps_b[:1, 0:OPN],
                                 axis=mybir.AxisListType.X)
            t_nmx = sb.tile([1, 1], F32)
            nc.vector.tensor_scalar_mul(t_nmx[:1, :], t_mx[:1, :], -1.0)
            t_e = sb.tile([1, OPN], F32)
            nc.scalar.activation(t_e[:1, :], ps_b[:1, 0:OPN], AF.Exp,
                                 bias=t_nmx[:1, 0:1])
            t_se = sb.tile([1, 1], F32)
            nc.vector.reduce_sum(t_se[:1, :], t_e[:1, :],
                                 axis=mybir.AxisListType.X)
            t_rs = sb.tile([1, 1], F32)
            nc.vector.reciprocal(t_rs[:1, :], t_se[:1, :])
            t_a = sb.tile([1, OPN], F32)
            nc.vector.tensor_scalar_mul(t_a[:1, :], t_e[:1, :], t_rs[:1, 0:1])

            # ---- coefficient vector [a0+a4, a1+a3, a2, -a3, -a4]
            t_cv = sb.tile([1, 5], F32)
            nc.vector.tensor_add(t_cv[:1, 0:1], t_a[:1, 0:1], t_a[:1, 4:5])
            nc.vector.tensor_add(t_cv[:1, 1:2], t_a[:1, 1:2], t_a[:1, 3:4])
            nc.vector.tensor_copy(t_cv[:1, 2:3], t_a[:1, 2:3])
            nc.vector.tensor_scalar_mul(t_cv[:1, 3:4], t_a[:1, 3:4], -1.0)
            nc.vector.tensor_scalar_mul(t_cv[:1, 4:5], t_a[:1, 4:5], -1.0)

            t_ne = sb.tile([1, MDIM], F32)
            nc.scalar.activation(t_ne[:1, :], ps_b[:1, OPN:OPN + MDIM],
                                 AF.Sigmoid)

            # broadcast coefficients to all 128 partitions via PE rank-1 trick
            ps_bc = pp.tile([128, 5], F32)
            nc.tensor.matmul(ps_bc[:, :], t_ones[:1, :].bitcast(F32),
                             t_cv[:1, :], start=True, stop=True)
            t_cf = sb.tile([128, 5], F32)
            nc.vector.tensor_copy(t_cf[:], ps_bc[:])

            # ---- memory shift-mix: (a0+a4)*next + (a1+a3)*prev + a2*mid
            t_t1 = sb.tile([128, QM * MDIM], F32)
            nc.vector.tensor_scalar_mul(t_t1[:], t_mmid[:], t_cf[:, 2:3])
            t_t2 = sb.tile([128, QM * MDIM], F32)
            nc.vector.scalar_tensor_tensor(t_t2[:], t_mnext[:], t_cf[:, 0:1],
                                           t_t1[:], op0=ALU.mult, op1=ALU.add)
            t_mo = sb.tile([128, QM * MDIM], F32)
            nc.vector.scalar_tensor_tensor(t_mo[:], t_mprev[:], t_cf[:, 1:2],
                                           t_t2[:], op0=ALU.mult, op1=ALU.add)

            # ---- boundary fixups (data-masked; no-ops on non-edge cores)
            # global row 0 (core 0, partition 0, chunk 0):
            #   += new_elt  - a3 * memory[MEM-1]
            nc.vector.scalar_tensor_tensor(
                t_mo[0:1, 0:MDIM], t_fixp[:1, :], t_cf[0:1, 3:4],
                t_mo[0:1, 0:MDIM], op0=ALU.mult, op1=ALU.add)
            t_nem = sb.tile([1, MDIM], F32)
            nc.vector.tensor_mul(t_nem[:1, :], t_ne[:1, :], t_nemask[:1, :])
            nc.vector.tensor_add(t_mo[0:1, 0:MDIM], t_mo[0:1, 0:MDIM],
                                 t_nem[:1, :])
            # global row MEM-1 (core 7, partition 127, last chunk):
            #   -= a4 * memory[0].  fixn is zero except its last row, so the
            #   op runs on the whole [96:128) partition block (DVE ops cannot
            #   start at partition 127) and is a no-op elsewhere.
            nc.vector.scalar_tensor_tensor(
                t_mo[96:128, (QM - 1) * MDIM:QM * MDIM],
                t_fixn[96:128, :], t_cf[96:128, 4:5],
                t_mo[96:128, (QM - 1) * MDIM:QM * MDIM],
                op0=ALU.mult, op1=ALU.add)

            for t in range(HT):
                i, j = divmod(t, CH_Y)
                nc.tensor.matmul(ps_a[:1, :], t_h[:, t:t + 1],
                                 t_wyt[i][:, j * BW:j * BW + SY],
                                 start=False, stop=(t == HT - 1))
            t_y = sb.tile([1, SY], F32)
            nc.scalar.activation(t_y[:1, :], ps_a[:1, :], AF.Sigmoid)
            dma(d_y[:], t_y[:1, :])

            dma(d_mout[:], t_mo[:])

    nc.finalize()
    return nc


_NC_CACHE = None


def _get_nc():
    global _NC_CACHE
    if _NC_CACHE is None:
        _NC_CACHE = build_nc()
    return _NC_CACHE


def make_in_maps(inputs):
    f = lambda a: np.ascontiguousarray(np.asarray(a), dtype=np.float32)
    x = f(inputs["input"]).reshape(VOC)
    h0 = f(inputs["hidden0"]).reshape(HID)
    memory = f(inputs["memory"])
    W_ih, b_ih = f(inputs["W_ih"]), f(inputs["b_ih"])
    W_hh, b_hh = f(inputs["W_hh"]), f(inputs["b_hh"])
    Wm_w, Wm_b = f(inputs["Wm_w"]), f(inputs["Wm_b"])
    Wy_w, Wy_b = f(inputs["Wy_w"]), f(inputs["Wy_b"])
    Wn_w, Wn_b = f(inputs["Wn_w"]), f(inputs["Wn_b"])
    Wa_w, Wa_b = f(inputs["Wa_w"]), f(inputs["Wa_b"])

    def ptile(a, cols):
        """(tiles*128, cols) -> [128, tiles*cols] partition-tiled layout."""
        n = a.shape[0] // 128
        return np.ascontiguousarray(
            a.reshape(n, 128, cols).transpose(1, 0, 2).reshape(128, n * cols))

    xp = np.ascontiguousarray(x.reshape(XT, 128).T)
    wihT = np.ascontiguousarray(W_ih.T)
    whhT = np.ascontiguousarray(W_hh.T)
    wyT = np.ascontiguousarray(Wy_w.T)
    waT = np.ascontiguousarray(Wa_w.T)
    wnT = np.ascontiguousarray(Wn_w.T)
    wm_tiled = ptile(Wm_w, MDIM)
    mem0b = np.ascontiguousarray(np.tile(memory[0], (128, 1)))
    bh_full = b_ih + b_hh
    bhb = np.ascontiguousarray((Wm_b + h0).reshape(HT, 128).T)
    zeros64 = np.zeros(MDIM, np.float32)
    ones64 = np.ones(MDIM, np.float32)
    fixn_r7 = np.zeros((32, MDIM), np.float32)
    fixn_r7[31] = memory[0]

    def rearr(a):
        return np.ascontiguousarray(
            a.reshape(QM, 128, MDIM).transpose(1, 0, 2).reshape(128, QM * MDIM))

    in_maps = []
    for r in range(NCORES):
        idx = _shard_index(r)
        rows = np.arange(r * SM, (r + 1) * SM)
        im = {
            "ones": np.ones((1, 128), np.float32),
            "xp": xp,
            "wih": ptile(wihT[:, idx], SH),
            "whh": ptile(whhT[:, idx], SH),
            "bh": bh_full[idx].reshape(1, SH),
            "wmt": wm_tiled,
            "mem0b": mem0b,
            "bhb": bhb,
            "wyt": ptile(np.concatenate(
                [wyT[:, r * SY:(r + 1) * SY], waT, wnT,
                 np.zeros((HID, 1), np.float32)], axis=1), BW),
            "by": np.concatenate(
                [Wy_b[r * SY:(r + 1) * SY], Wa_b, Wn_b,
                 np.zeros(1, np.float32)]).reshape(1, BW),
            "mmid": rearr(memory[rows]),
            "mnext": rearr(memory[(rows + 1) % MEM]),
            "mprev": rearr(memory[(rows - 1) % MEM]),
            "fixp": (memory[MEM - 1] if r == 0 else zeros64).reshape(1, MDIM),
            "fixn": fixn_r7 if r == NCORES - 1 else np.zeros((32, MDIM),
                                                            np.float32),
            "nemask": (ones64 if r == 0 else zeros64).reshape(1, MDIM),
        }
        in_maps.append({k: np.ascontiguousarray(v, dtype=np.float32)
                        for k, v in im.items()})
    return in_maps


def assemble_outputs(results):
    y = np.concatenate(
        [results[r]["y"].reshape(SY) for r in range(NCORES)]).reshape(1, 1, OUT)
    ht = np.empty(HID, np.float32)
    for r in range(NCORES):
        ht[_shard_index(r)] = results[r]["hc"].reshape(SH)
    ht = ht.reshape(1, 1, HID)
    mem = np.concatenate(
        [results[r]["mout"].reshape(128, QM, MDIM).transpose(1, 0, 2)
         .reshape(SM, MDIM) for r in range(NCORES)], axis=0)
    return (np.ascontiguousarray(y, np.float32),
            np.ascontiguousarray(ht, np.float32),
            np.ascontiguousarray(mem, np.float32))


def kernel(**inputs):
    nc = _get_nc()
    in_maps = make_in_maps(inputs)
    res = run_bass_kernel_spmd(nc, in_maps, list(range(NCORES))).results
    return assemble_outputs(res)
